# revision 22
# baseline (speedup 1.0000x reference)
"""BEiT-style windowed attention with relative position bias, on 8 trn2 cores.

Sharding: data-parallel over batch (32 batches -> 4 per core). Weights and the
host-gathered rel-pos bias are replicated.

DMA strategy (per-dma_start fixed cost is ~2.6us, so transfers are batched
and spread across issuing rings): x/zr/output rows ride the sync HWDGE ring,
weights + fp8 bias ride the gpsimd SWDGE ring, staged outputs the scalar
ring. The fp8 rel-pos bias ships as raw bytes inside the bf16 mega image
(finite e4m3 byte pairs can never alias a bf16 NaN/Inf pattern).

Device pipeline per core (all matmuls bf16/fp8 in, f32 accumulate):
  Phase 1: batch-major QKV projections, v tiles interleaved between qkT
           pairs so psum recycling hides the evacuation latency.
  Phase 2: per (head, batch): six 1-bank score psum tiles (5 c0 key-chunks +
           c1 tail); per key-chunk an ident-matmul accumulates the fp8 bias,
           then the QK matmul adds scores; per-chunk exp on ScalarE writes E
           (bf16) as soon as that chunk lands. PV (stationary [v|1], softmax
           denominators on row 64) runs TWO iterations behind QK so no FIFO
           head-of-line blocking on the exp chain. Per head: single-copy
           evacuation into a [65, 4*577] f32 image, then one row-64->0 DMA,
           one partition_broadcast, one reciprocal, one normalize multiply.
  Phase 3: out^T = Wp^T.T @ O^T + b staged per m-tile -> 6 fp16 DMAs out.
"""

import numpy as np
import ml_dtypes

import concourse.bass as bass
import concourse.tile as tile
from concourse import bacc, mybir
from concourse.bass_utils import run_bass_kernel_spmd

BF16 = mybir.dt.bfloat16
FP8 = mybir.dt.float8e4
F16 = mybir.dt.float16
F32 = mybir.dt.float32
AF = mybir.ActivationFunctionType

NCORES = 8
B = 32
BPC = B // NCORES          # batches per core
N = 577                    # sequence length
C = 768
H = 12
HD = 64
R = BPC * N                # rows per core (2308)
CT = C // 128              # 6 contraction tiles
MT = 12                    # qk output row-tiles (1536/128)
JTS = [128, 128, 128, 128, 65]   # j tiles of N
ECOLS = 2885               # packed score/E columns: 5*512 + 5*65
BCOLS = 2886               # fp8 bias cols per head (ECOLS + 1 pad, even)
BHC = BCOLS // 2           # 1443 bf16 transport cols per head
RCHUNKS = [(0, 512), (512, 512), (1024, 512), (1536, 512), (2048, 260)]

# mega buffer column offsets (bf16 image [128, MEGA_COLS]).
XBB = CT * N                        # 3462 cols per batch block
XT_OFF = 0
WQK_OFF = XT_OFF + BPC * XBB        # 13848
WV_OFF = WQK_OFF + MT * CT * 128    # 23064
WP_OFF = WV_OFF + CT * C            # 27672
PBH_OFF = WP_OFF + CT * C           # 32280
PBL_OFF = PBH_OFF + CT              # 32286
ID_OFF = PBL_OFF + CT               # 32292
BIAS_OFF = ID_OFF + 128             # 32420
MEGA_COLS = BIAS_OFF + H * BHC      # 49736

_PROGRAM = None


def build_program(reps=1, phases=(1, 2, 3)):
    """reps > 1 unrolls the whole kernel body that many times in one NEFF —
    used only by test.py. phases: ablation switch for timing experiments."""
    nc = bacc.Bacc(trn_type="TRN2", name="beit_attn")

    mega_d = nc.dram_tensor("mega", [128, MEGA_COLS], BF16, kind="ExternalInput")
    out_d = nc.dram_tensor("ftout", [128, CT * R], F16, kind="ExternalOutput")

    with tile.TileContext(nc) as tc:
        for _rep in range(reps):
            _build_body(nc, tc, mega_d, out_d, phases)

    nc.compile()
    return nc


def _build_body(nc, tc, mega_d, out_d, phases=(1, 2, 3)):
    NG = 4                  # bias head-groups of 3
    with (
            tc.tile_pool(name="static", bufs=1) as sp,
            tc.tile_pool(name="qk", bufs=1) as qk_pool,
            tc.tile_pool(name="v1", bufs=1) as v1_pool,
        ):
            pbhl = sp.tile([128, 2 * CT], BF16, tag="pbhl")
            nc.sync.dma_start(pbhl[:], mega_d[:, PBH_OFF : PBH_OFF + 2 * CT])
            pb = sp.tile([128, CT], F32, tag="pb")
            nc.vector.tensor_add(pb[:, :], pbhl[:, 0:CT], pbhl[:, CT : 2 * CT])
            ident = sp.tile([128, 128], BF16, tag="ident")
            nc.sync.dma_start(ident[:], mega_d[:, ID_OFF : ID_OFF + 128])
            ot_sb = sp.tile([128, CT * R], BF16, tag="ot")
            # fp8 bias head-groups (3 heads per DMA), double-buffered; group
            # 0 staged here so its (SWDGE) DMA overlaps phase 1.
            bg = [sp.tile([128, 3 * BHC], BF16, tag=f"bg{i}", name=f"bg{i}") for i in range(2)]

            def bias_dma(g):
                nc.gpsimd.dma_start(
                    bg[g % 2][:], mega_d[:, BIAS_OFF + 3 * BHC * g : BIAS_OFF + 3 * BHC * (g + 1)]
                )

            def bias_src(h):
                t = bg[(h // 3) % 2]
                return t[:, (h % 3) * BHC : (h % 3 + 1) * BHC].bitcast(FP8)

            if 2 in phases:
                bias_dma(0)

            qk_t = [qk_pool.tile([128, R], BF16, tag=f"qk{m}", name=f"qk{m}") for m in range(MT)]
            v1_t = [
                [v1_pool.tile([128, 780], BF16, tag=f"v1_{b}_{t}", name=f"v1_{b}_{t}") for t in range(5)]
                for b in range(BPC)
            ]

            # ---------------- Phase 1: QKV projections -------------------
            with (
                tc.tile_pool(name="ph1", bufs=1) as p1,
                tc.tile_pool(name="psq", bufs=2, space="PSUM") as psq,
                tc.tile_pool(name="psv", bufs=2, space="PSUM") as psv,
            ):
                wqk_sb = p1.tile([128, MT * CT * 128], BF16, tag="wqk")
                wv_sb = p1.tile([128, CT * C], BF16, tag="wv")
                if 1 in phases:
                    # weights on the SWDGE ring (x + outputs own the sync ring)
                    for i in range(3):
                        nc.gpsimd.dma_start(
                            wqk_sb[:, 3072 * i : 3072 * (i + 1)],
                            mega_d[:, WQK_OFF + 3072 * i : WQK_OFF + 3072 * (i + 1)],
                        )
                    nc.gpsimd.dma_start(wv_sb[:], mega_d[:, WV_OFF : WV_OFF + CT * C])
                xt_b = []
                for b in range(BPC if 1 in phases else 0):
                    xb = p1.tile([128, XBB], BF16, tag=f"xtb{b}")
                    nc.sync.dma_start(
                        xb[:], mega_d[:, XT_OFF + XBB * b : XT_OFF + XBB * (b + 1)]
                    )
                    xt_b.append(xb)

                def emit_qkT(m, b):
                    # 512 + 65 column groups: a matmul output may not cross a
                    # PSUM bank boundary (2 KB = 512 f32). kk-outer so both
                    # groups' step kk share one stationary load.
                    ps = psq.tile([128, 1024], F32, tag="psq")
                    for kk in range(CT):
                        for c0, cn in ((0, 512), (512, 65)):
                            nc.tensor.matmul(
                                ps[:, c0 : c0 + cn],
                                wqk_sb[:, 768 * m + 128 * kk : 768 * m + 128 * (kk + 1)],
                                xt_b[b][:, N * kk + c0 : N * kk + c0 + cn],
                                start=(kk == 0),
                                stop=(kk == CT - 1),
                            )
                    if (m + b) % 2 == 0:
                        nc.vector.tensor_copy(qk_t[m][:, N * b : N * (b + 1)], ps[:, 0:N])
                    else:
                        nc.scalar.copy(qk_t[m][:, N * b : N * (b + 1)], ps[:, 0:N])

                def emit_v(b, t):
                    kj = JTS[t]
                    # both vc groups in one 2-bank tile, kk-outer so each
                    # x stationary is loaded once per kk
                    ps = psv.tile([128, 768], F32, tag="psv")
                    for kk in range(CT):
                        for vc0, vcn in ((0, 512), (512, 256)):
                            nc.tensor.matmul(
                                ps[:kj, vc0 : vc0 + vcn],
                                xt_b[b][:, N * kk + 128 * t : N * kk + 128 * t + kj],
                                wv_sb[:, C * kk + vc0 : C * kk + vc0 + vcn],
                                start=(kk == 0),
                                stop=(kk == CT - 1),
                            )
                    for vc0, vcn, h0 in ((0, 512, 0), (512, 256, 8)):
                        dest = v1_t[b][t][:kj, 65 * h0 : 65 * (h0 + vcn // 64)]
                        dest = dest.rearrange("p (h d) -> p h d", d=65)[:, :, 0:64]
                        src = ps[:kj, vc0 : vc0 + vcn].rearrange("p (h d) -> p h d", d=64)
                        nc.vector.tensor_copy(dest, src)
                    ones_ap = v1_t[b][t][:, :].rearrange("p (h d) -> p h d", d=65)[
                        :, :, 64:65
                    ]
                    nc.gpsimd.memset(ones_ap, 1.0)

                for b in range(BPC if 1 in phases else 0):
                    # v tiles interleaved between qkT pairs: the tensor queue
                    # always has non-psq work between two psq users, hiding
                    # the evacuation latency of the recycled psum.
                    for m in range(MT):
                        emit_qkT(m, b)
                        if m % 2 == 1 and m // 2 < 5:
                            emit_v(b, m // 2)

            # ---------------- Phases 2+3 share the wp pool ---------------
            with tc.tile_pool(name="wpp", bufs=1) as wpp:
                wp_sb = wpp.tile([128, CT * CT * 128], BF16, tag="wp")
                if 3 in phases:
                    nc.gpsimd.dma_start(wp_sb[:], mega_d[:, WP_OFF : WP_OFF + CT * C])

                # ---------------- Phase 2: attention ---------------------
                # Software pipeline: iteration t emits QK(t), then PV/evac
                # for t-2, then the per-chunk exps. The 2-iteration lag keeps
                # the tensor/DVE FIFOs from head-of-line blocking on the exp
                # chain; six 1-bank score tiles let each exp start as soon as
                # its chunk lands, so the psum-reuse cycle is per-bank.
                with (
                    tc.tile_pool(name="ph2", bufs=2) as p2,
                    tc.tile_pool(name="ph2e", bufs=3) as p2e,
                    tc.tile_pool(name="ph2t", bufs=1) as p2t,
                    tc.tile_pool(name="score", bufs=1, space="PSUM") as sc_pool,
                    tc.tile_pool(name="pvp", bufs=1, space="PSUM") as pv_pool,
                ):
                    def emit_qk(h, b):
                        # Two 3-bank score tiles: SA = jt0/1/2 c0 (E cols
                        # 0:1536), SB = jt3/4 c0 + the packed c1 tail (E cols
                        # 1536:2885). Two exps per iteration — PSUM-source
                        # ACT pays ~280ns fixed per instruction, so fewer,
                        # bigger activations win; the A/B split still lets
                        # QK(t+1)'s first half start after exp_A(t) only.
                        qp = 64 * (h % 2)
                        qm, km = h // 2, 6 + h // 2
                        bt = bias_src(h)
                        SA = sc_pool.tile([128, 1536], F32, tag="SA")
                        SB = sc_pool.tile([128, 1536], F32, tag="SB")
                        q0 = qk_t[qm][qp : qp + 64, N * b : N * b + 512]
                        q1 = qk_t[qm][qp : qp + 64, N * b + 512 : N * b + 577]

                        def kT(jt, kj):
                            return qk_t[km][qp : qp + 64, N * b + 128 * jt : N * b + 128 * jt + kj]

                        chain_prev = None

                        def mm(out, lhsT, rhs, start, stop, chain=False):
                            nonlocal chain_prev
                            m_ = nc.tensor.matmul(
                                out, lhsT, rhs, start=start, stop=stop,
                                skip_group_check=chain,
                            )
                            if chain:
                                if chain_prev is not None:
                                    tile.add_dep_helper(m_.ins, chain_prev.ins, reason="c1 chain")
                                chain_prev = m_
                            return m_

                        # half A: bias for jt0-2 (one ident load), then QK
                        for jt in range(3):
                            mm(SA[:, 512 * jt : 512 * jt + 512], ident[:, :],
                               bt[:, 512 * jt : 512 * jt + 512], True, False)
                        for jt in range(3):
                            mm(SA[:128, 512 * jt : 512 * jt + 512], kT(jt, 128), q0,
                               False, True)
                        # half B: bias jt3/4 c0 + all c1 (one ident load),
                        # then QK. start=True clears has_written for the
                        # WHOLE bank, so the c1 region gets ONE bias matmul
                        # (single accumulation group over all 325 cols); the
                        # five QK matmuls then accumulate into sub-ranges.
                        # jt4 c0 mixes M=128 bias with M=65 QK -> chain.
                        mm(SB[:, 0:512], ident[:, :], bt[:, 1536:2048], True, False)
                        mm(SB[:, 512:1024], ident[:, :], bt[:, 2048:2560], True, False,
                           chain=True)
                        mm(SB[:, 1024:1349], ident[:, :], bt[:, 2560:2885], True, False,
                           chain=True)
                        mm(SB[:128, 0:512], kT(3, 128), q0, False, True)
                        mm(SB[:65, 512:1024], kT(4, 65), q0, False, True, chain=True)
                        for jt in range(5):
                            kj = JTS[jt]
                            mm(SB[:kj, 1024 + 65 * jt : 1024 + 65 * jt + 65],
                               kT(jt, kj), q1, False, True, chain=True)
                        return SA, SB

                    def emit_softmax(h, b, SA, SB):
                        E_t = p2e.tile([128, ECOLS], BF16, tag="E")
                        nc.scalar.activation(E_t[:, 0:1536], SA[:, :], AF.Exp)
                        nc.scalar.activation(E_t[:, 1536:ECOLS], SB[:, 0 : ECOLS - 1536], AF.Exp)
                        return E_t

                    ocf = {}

                    def emit_pv(h, b, E_t):
                        pv = pv_pool.tile([128, 1024], F32, tag="pv")
                        for jt in range(5):
                            kj = JTS[jt]
                            vst = v1_t[b][jt][:kj, 65 * h : 65 * h + 65]
                            nc.tensor.matmul(
                                pv[0:65, 0:512], vst, E_t[:kj, 512 * jt : 512 * jt + 512],
                                start=(jt == 0), stop=(jt == 4),
                            )
                            nc.tensor.matmul(
                                pv[0:65, 512:577], vst,
                                E_t[:kj, 2560 + 65 * jt : 2560 + 65 * jt + 65],
                                start=(jt == 0), stop=(jt == 4),
                            )
                        # Single-copy evacuation (rows 0-63 = head output,
                        # row 64 = softmax denominator) into the per-head f32
                        # staging image; frees pv for the next iteration.
                        if b == 0:
                            ocf[h] = p2.tile([65, R], F32, tag="ocf", name=f"ocf{h}")
                        nc.vector.tensor_copy(ocf[h][:, N * b : N * (b + 1)], pv[0:65, 0:N])

                    def emit_head_tail(h):
                        # Batched softmax tail for all 4 batches of head h.
                        # partition_broadcast reads absolute partition 0: DMA
                        # the denominator row 64 -> 0 first.
                        zr = p2t.tile([1, R], F32, tag="zr")
                        nc.sync.dma_start(zr[0:1, :], ocf[h][64:65, :])
                        rb = p2t.tile([64, R], F32, tag="rb")
                        nc.gpsimd.partition_broadcast(rb[:, :], zr[0:1, :], channels=64)
                        nc.vector.reciprocal_approx_fast(rb[:, :], rb[:, :])
                        if h % 2 == 0:
                            nc.vector.tensor_mul(
                                ot_sb[0:64, R * (h // 2) : R * (h // 2) + R],
                                ocf[h][0:64, :],
                                rb[:, :],
                            )
                        else:
                            stg = p2t.tile([64, R], BF16, tag="stg")
                            nc.vector.tensor_mul(stg[:, :], ocf[h][0:64, :], rb[:, :])
                            nc.scalar.dma_start(
                                ot_sb[64:128, R * (h // 2) : R * (h // 2) + R], stg[:, :]
                            )
                        del ocf[h]

                    pend = []
                    steps = [(h, b) for h in range(H) for b in range(BPC)] if 2 in phases else []

                    def drain_one():
                        ph, pb_, pE = pend.pop(0)
                        emit_pv(ph, pb_, pE)
                        if pb_ == BPC - 1:
                            emit_head_tail(ph)

                    for h, b in steps:
                        if b == 0 and h % 3 == 0 and h // 3 + 1 < NG:
                            bias_dma(h // 3 + 1)
                        Ss, SC = emit_qk(h, b)
                        if len(pend) >= 2:
                            drain_one()
                        pend.append((h, b, emit_softmax(h, b, Ss, SC)))
                    while pend:
                        drain_one()

                # ---------------- Phase 3: output projection -------------
                with (
                    tc.tile_pool(name="ph3", bufs=2) as p3,
                    tc.tile_pool(name="psum3", bufs=8, space="PSUM") as psum3,
                ):
                    for m in range(CT if 3 in phases else 0):
                        # kk-outer across all 5 row chunks: each wp stationary
                        # is loaded once per kk and streams 2308 columns
                        pss = [
                            psum3.tile([128, 512], F32, tag="ps3", name=f"ps3_{m}_{ci}")
                            for ci in range(len(RCHUNKS))
                        ]
                        for kk in range(CT):
                            for ps, (c0, cn) in zip(pss, RCHUNKS):
                                nc.tensor.matmul(
                                    ps[:, :cn],
                                    wp_sb[:, 768 * m + 128 * kk : 768 * m + 128 * (kk + 1)],
                                    ot_sb[:, R * kk + c0 : R * kk + c0 + cn],
                                    start=(kk == 0),
                                    stop=(kk == CT - 1),
                                )
                        ft = p3.tile([128, R], F16, tag="ft")
                        for ps, (c0, cn) in zip(pss, RCHUNKS):
                            nc.scalar.add(ft[:, c0 : c0 + cn], ps[:, :cn], pb[:, m : m + 1])
                        nc.sync.dma_start(out_d[:, R * m : R * (m + 1)], ft[:, :])


def get_program():
    global _PROGRAM
    if _PROGRAM is None:
        _PROGRAM = build_program()
    return _PROGRAM


def make_host_inputs(x, qkv_w, table, rel_index, proj_w, proj_b):
    bf = ml_dtypes.bfloat16
    f8 = ml_dtypes.float8_e4m3
    x = np.asarray(x, np.float32)
    qkv_w = np.asarray(qkv_w, np.float32)
    table = np.asarray(table, np.float32)
    rel_index = np.asarray(rel_index)
    proj_w = np.asarray(proj_w, np.float32)
    proj_b = np.asarray(proj_b, np.float32)

    qkv_ws = qkv_w.copy()
    qkv_ws[:768] *= 0.125                                    # fold q scale (exact in bf16)
    wt = qkv_ws.T                                            # [768, 2304]
    # qk half m-major: wqk[p, m*768 + kk*128 + cc] = wt[kk*128+p, m*128+cc]
    wqk = np.ascontiguousarray(
        wt[:, :1536].reshape(CT, 128, MT, 128).transpose(1, 2, 0, 3).reshape(128, MT * CT * 128)
    ).astype(bf)
    # v half kk-major: wv[p, kk*768 + vc] = wt[kk*128+p, 1536+vc]
    wv = np.ascontiguousarray(
        wt[:, 1536:].reshape(CT, 128, C).transpose(1, 0, 2).reshape(128, CT * C)
    ).astype(bf)
    # m-major: wp[p, m*768 + kk*128 + cc] = proj_w.T[kk*128+p, m*128+cc]
    wp = np.ascontiguousarray(
        proj_w.T.reshape(CT, 128, CT, 128).transpose(1, 2, 0, 3).reshape(128, CT * CT * 128)
    ).astype(bf)                                             # [128, 6*768]
    pb_hi = proj_b.astype(bf)
    pb_lo = (proj_b - pb_hi.astype(np.float32)).astype(bf)
    pbh = np.ascontiguousarray(pb_hi.reshape(CT, 128).T)     # [128, 6]
    pbl = np.ascontiguousarray(pb_lo.reshape(CT, 128).T)
    ident = np.eye(128, dtype=bf)

    # bias, transposed orientation: biasT[h, j, i] = table[rel_index[i, j], h]
    g = table[rel_index.reshape(-1)].reshape(N, N, H)        # [i, j, h]
    bt = g.transpose(2, 1, 0)                                # [h, j, i]
    btp = np.zeros((H, 640, N), np.float32)
    btp[:, :N] = bt
    btp = btp.reshape(H, 5, 128, N)
    c0 = btp[:, :, :, 0:512].transpose(0, 2, 1, 3).reshape(H, 128, 2560)
    c1 = btp[:, :, :, 512:577].transpose(0, 2, 1, 3).reshape(H, 128, 325)
    pad = np.zeros((H, 128, BCOLS - ECOLS), np.float32)
    biasp = np.ascontiguousarray(
        np.concatenate([c0, c1, pad], axis=2)
    ).astype(f8)                                             # [12, 128, 2886]
    # fp8 bias bytes ride in the bf16 image, two per column
    bias_bf = (
        biasp.transpose(1, 0, 2).reshape(128, H * BCOLS).view(np.uint16).view(bf)
    )                                                        # [128, 12*1443]

    shared = np.concatenate([wqk, wv, wp, pbh, pbl, ident, bias_bf], axis=1)
    in_maps = []
    for c in range(NCORES):
        xT = x[BPC * c : BPC * (c + 1)].reshape(R, C).T      # [768, 2308]
        # batch-major x image: xtb[p, b*XBB + kk*N + j] = xT[kk*128+p, b*N+j]
        xtb = np.ascontiguousarray(
            xT.reshape(CT, 128, BPC, N).transpose(1, 2, 0, 3).reshape(128, BPC * XBB)
        ).astype(bf)
        mega = np.concatenate([xtb, shared], axis=1)
        in_maps.append({"mega": np.ascontiguousarray(mega)})
    return in_maps


def unpack_output(ft):
    """[128, 6*2308] f16 -> [BPC, 577, 768] f32."""
    f = np.asarray(ft, np.float32).reshape(128, CT, R).transpose(1, 0, 2).reshape(C, R)
    return np.ascontiguousarray(f.T).reshape(BPC, N, C)


def kernel(x, qkv_w, table, rel_index, proj_w, proj_b):
    nc = get_program()
    in_maps = make_host_inputs(x, qkv_w, table, rel_index, proj_w, proj_b)
    res = run_bass_kernel_spmd(nc, in_maps, core_ids=list(range(NCORES)))
    out = np.empty((B, N, C), np.float32)
    for c in range(NCORES):
        out[BPC * c : BPC * (c + 1)] = unpack_output(res.results[c]["ftout"])
    return out
